# revision 15
# baseline (speedup 1.0000x reference)
"""Trainium2 Bass kernel for NeighborsValuesAssigner (retrieval_knn).

out[b,:,h,w] = mean_{n in top8} values[n]  where top8 = 8 smallest
dist[b,n,h,w] = 0.5||p_n||^2 - <p_n, x_patch(b,h,w)>  (5x5 'same' conv).

8 cores, data-parallel over batch (4 images/core). Per core:
  pass1  score[px,n] = <p_n,x_px> - 0.5||p_n||^2 on PE as 3 accumulating
         fp16 matmuls (hi/lo fp16 split: xh@ph + xh@pl + xl@ph; error ~2^-22).
  top8   DVE max8 over each [128,1024] PSUM half + merge -> t8 = 8th value.
  bias   GPSIMD: negt8 = -t8 + eps  (eps=1e-5 makes the boundary element
         strictly positive; elements with gap<eps are rare, see analysis).
  mask   ACT Sign(score + negt8) straight from PSUM -> f16 {+1,-1} mask
         [px, n].  (+1 for top-8, -1 for the rest.)
  maskT  DMA-engine transpose (dma_start_transpose) [px,128n] -> [n,128px]
         per chunk: PE never touches transposes, HAM stays warm.
  matmul outpm[D,px] = sum_n values[n,D]*maskT[n,px]  = 2*S_top8 - S_all
         (fp16 operands, fp32 PSUM accumulation over 16 chunks of n).
  final  ACT: out = outpm*(1/16) + colsum/16  (= mean of top-8) -> DMA out.
"""
import sys

sys.path.insert(0, "/opt/trn_rl_repo")

import numpy as np
import ml_dtypes

B, C, H, W = 32, 3, 64, 64
N, D = 2048, 128
KH = KW = 5
KDIM = C * KH * KW          # 75
KROWS = KDIM + 1            # 76 = patch dims + bias/ones row
NCORES = 8
BLOC = B // NCORES          # 4 images per core
PX = BLOC * H * W           # 16384 pixels per core
GPX = 512                   # pixels per group
NGRP = PX // GPX            # 32 groups per core
NCHUNK = N // 128           # 16 patch chunks
EPS = 1e-5

_CACHE = {}


def _build_program(loop_r=0, ablate=()):
    """loop_r=0: straight-line. loop_r>0: wrap body in a device-side
    For_i loop running it loop_r times (for HW timing via wall deltas).
    ablate: subset of {"dve","act","gpsimd","tp","vmm"} — drop those stages
    (breaks correctness; for timing experiments only)."""
    import concourse.bacc as bacc
    import concourse.tile as tile
    import concourse.mybir as mybir
    from contextlib import ExitStack

    f32 = mybir.dt.float32
    f16 = mybir.dt.float16
    nc = bacc.Bacc("TRN2", target_bir_lowering=False, debug=False)

    xph = nc.dram_tensor("xph", [KROWS, PX], f16, kind="ExternalInput").ap()
    xpl = nc.dram_tensor("xpl", [KROWS, PX], f16, kind="ExternalInput").ap()
    ph = nc.dram_tensor("ph", [KROWS, N], f16, kind="ExternalInput").ap()
    pl = nc.dram_tensor("pl", [KROWS, N], f16, kind="ExternalInput").ap()
    vs16 = nc.dram_tensor("vs16", [128, N], f16, kind="ExternalInput").ap()
    cs16 = nc.dram_tensor("cs16", [128, 1], f32, kind="ExternalInput").ap()
    out = nc.dram_tensor("out", [BLOC, 128, H * W], f32, kind="ExternalOutput").ap()

    SIGN = mybir.ActivationFunctionType.Sign
    IDENT = mybir.ActivationFunctionType.Identity

    with tile.TileContext(nc) as tc, ExitStack() as ctx:
        const = ctx.enter_context(tc.tile_pool(name="const", bufs=1))
        xpp = ctx.enter_context(tc.tile_pool(name="xpp", bufs=3))
        mhp = ctx.enter_context(tc.tile_pool(name="mhp", bufs=4))
        mkp = ctx.enter_context(tc.tile_pool(name="mkp", bufs=6))
        mtp = ctx.enter_context(tc.tile_pool(name="mtp", bufs=2))
        otp = ctx.enter_context(tc.tile_pool(name="otp", bufs=2))
        ps1 = ctx.enter_context(tc.tile_pool(name="ps1", bufs=6, space="PSUM"))
        psB = ctx.enter_context(tc.tile_pool(name="psB", bufs=2, space="PSUM"))

        ph_t = const.tile([KROWS, N], f16)
        pl_t = const.tile([KROWS, N], f16)
        vs_t = const.tile([128, N], f16)
        cs_t = const.tile([128, 1], f32)
        nc.sync.dma_start(ph_t[:], ph[:])
        nc.sync.dma_start(pl_t[:], pl[:])
        nc.sync.dma_start(vs_t[:], vs16[:])
        nc.sync.dma_start(cs_t[:], cs16[:])

        loop_cm = tc.For_i(0, loop_r, 1) if loop_r else None
        if loop_cm is not None:
            loop_cm.__enter__()

        grp_per_img = (H * W) // GPX  # 8

        def pass1_tile(t, xh_t, xl_t, mt):
            lh = xh_t[:, t * 128:(t + 1) * 128]
            ll = xl_t[:, t * 128:(t + 1) * 128]
            mh = mhp.tile([128, 32], f32, tag="mh")
            m8 = mhp.tile([128, 8], f32, tag="m8")
            ng = mhp.tile([128, 1], f32, tag="ng")
            mk = mkp.tile([128, N], f16, tag="mk")
            quads = []
            for q in range(4):  # N=512 per matmul (one PSUM bank)
                rsl = slice(q * 512, (q + 1) * 512)
                p1 = ps1.tile([128, 512], f32, tag="p1")
                nc.tensor.matmul(p1[:], lh, ph_t[:, rsl],
                                 start=True, stop=False)
                nc.tensor.matmul(p1[:], lh, pl_t[:, rsl],
                                 start=False, stop=False)
                nc.tensor.matmul(p1[:], ll, ph_t[:, rsl],
                                 start=False, stop=True)
                if "dve" not in ablate:
                    nc.vector.max(mh[:, q * 8:(q + 1) * 8], p1[:])
                quads.append(p1)
            if "dve" not in ablate:
                nc.vector.max(m8[:], mh[:])
                nc.vector.tensor_scalar(ng[:], m8[:, 7:8], -1.0, EPS,
                                        mybir.AluOpType.mult,
                                        mybir.AluOpType.add)
            if "act" not in ablate:
                for q in range(4):
                    nc.scalar.activation(mk[:, q * 512:(q + 1) * 512],
                                         quads[q][:], SIGN,
                                         bias=ng[:, 0:1], scale=1.0)
            if "tp" not in ablate:
                nc.sync.dma_start_transpose(
                    mt[:, :, t * 128:(t + 1) * 128], mk[:, :])

        # values-matmul chunks of group g-1 interleaved after px-tiles
        # 1,2,3 of group g (chunk 0 carries start=True, 15 carries stop).
        VSPREAD = {1: range(0, 5), 2: range(5, 10), 3: range(10, 16)}

        def vchunks(pB, mt, cs):
            if "vmm" in ablate:
                return
            for c in cs:
                rhs = (vs_t[:, 0:GPX] if "tp" in ablate else mt[:, c, :])
                nc.tensor.matmul(
                    pB[:], vs_t[:, c * 128:(c + 1) * 128], rhs,
                    start=(c == 0), stop=(c == NCHUNK - 1))

        def finish(g, pB):
            b, s = divmod(g, grp_per_img)
            ot = otp.tile([128, GPX], f32, tag="ot")
            nc.scalar.activation(ot[:], pB[:], IDENT,
                                 bias=cs_t[:, 0:1], scale=0.0625)
            nc.sync.dma_start(out[b, :, s * GPX:(s + 1) * GPX], ot[:])

        prev = None  # (g-1, pB, mt)
        for g in range(NGRP):
            xh_t = xpp.tile([KROWS, GPX], f16, tag="xh")
            xl_t = xpp.tile([KROWS, GPX], f16, tag="xl")
            nc.sync.dma_start(xh_t[:], xph[:, g * GPX:(g + 1) * GPX])
            nc.sync.dma_start(xl_t[:], xpl[:, g * GPX:(g + 1) * GPX])
            mt = None if "tp" in ablate else mtp.tile([128, NCHUNK, GPX], f16, tag="mt")
            for t in range(4):
                pass1_tile(t, xh_t, xl_t, mt)
                if prev is not None and t in VSPREAD:
                    vchunks(prev[1], prev[2], VSPREAD[t])
            if prev is not None:
                finish(prev[0], prev[1])
            pB = psB.tile([128, GPX], f32, tag="pB")
            prev = (g, pB, mt)
        vchunks(prev[1], prev[2], range(NCHUNK))
        finish(prev[0], prev[1])

        if loop_cm is not None:
            loop_cm.__exit__(None, None, None)

    nc.compile()
    return nc


def _get_program():
    if "nc" not in _CACHE:
        _CACHE["nc"] = _build_program()
    return _CACHE["nc"]


def _im2col(x):
    """x: (B,3,64,64) f32 -> cols (B, 75, 4096) f32, k=(c,dy,dx), px=(h,w)."""
    xpad = np.pad(x, ((0, 0), (0, 0), (2, 2), (2, 2)))
    win = np.lib.stride_tricks.sliding_window_view(xpad, (KH, KW), axis=(2, 3))
    cols = np.ascontiguousarray(win.transpose(0, 1, 4, 5, 2, 3))
    return cols.reshape(x.shape[0], KDIM, H * W)


def _host_prep(x, patches, values):
    """Returns per-core in_maps list."""
    pf = patches.reshape(N, KDIM)
    bias = (-0.5 * np.sum(pf.astype(np.float64) ** 2, axis=1)).astype(np.float32)

    pfull = np.zeros((KROWS, N), np.float32)
    pfull[0:KDIM] = pf.T
    pfull[KDIM] = bias
    ph = pfull.astype(np.float16)
    pl = (pfull - ph.astype(np.float32)).astype(np.float16)

    vs16 = np.ascontiguousarray(
        values.reshape(NCHUNK, 128, 128).transpose(1, 0, 2).reshape(128, N)
    ).astype(np.float16)
    # cs16[d] = sum_n values[n, d] / 16 : vs16[r, c*128+d] = values[c*128+r, d]
    cs16 = (vs16.astype(np.float64).reshape(128, NCHUNK, 128).sum(axis=(0, 1))
            / 16.0).astype(np.float32).reshape(128, 1)

    cols = _im2col(x)  # (32, 75, 4096) f32
    in_maps = []
    for i in range(NCORES):
        xfull = np.empty((KROWS, PX), np.float32)
        xfull[0:KDIM] = np.concatenate(
            [cols[i * BLOC + j] for j in range(BLOC)], axis=1)
        xfull[KDIM] = 1.0
        xh = xfull.astype(np.float16)
        xl = (xfull - xh.astype(np.float32)).astype(np.float16)
        in_maps.append({"xph": xh, "xpl": xl, "ph": ph, "pl": pl,
                        "vs16": vs16, "cs16": cs16})
    return in_maps


def kernel(x, patches, values):
    from concourse.bass_utils import run_bass_kernel_spmd

    x = np.asarray(x, dtype=np.float32)
    patches = np.asarray(patches, dtype=np.float32)
    values = np.asarray(values, dtype=np.float32)

    nc = _get_program()
    in_maps = _host_prep(x, patches, values)
    res = run_bass_kernel_spmd(nc, in_maps, list(range(NCORES)))

    out = np.empty((B, D, H, W), np.float32)
    for i in range(NCORES):
        o = res.results[i]["out"]  # (BLOC, 128, 4096)
        out[i * BLOC:(i + 1) * BLOC] = o.reshape(BLOC, D, H, W)
    return out


# revision 19
# speedup vs baseline: 1.0906x; 1.0906x over previous
"""Trainium2 Bass kernel for NeighborsValuesAssigner (retrieval_knn).

out[b,:,h,w] = mean_{n in top8} values[n]  where top8 = 8 smallest
dist[b,n,h,w] = 0.5||p_n||^2 - <p_n, x_patch(b,h,w)>  (5x5 'same' conv).

8 cores, data-parallel over batch (4 images/core). Per core:
  pass1  score[px,n] = <p_n,x_px> - 0.5||p_n||^2 on PE as 3 accumulating
         fp16 matmuls (hi/lo fp16 split: xh@ph + xh@pl + xl@ph; error ~2^-22).
  top8   DVE max8 over each [128,1024] PSUM half + merge -> t8 = 8th value.
  bias   GPSIMD: negt8 = -t8 + eps  (eps=1e-5 makes the boundary element
         strictly positive; elements with gap<eps are rare, see analysis).
  mask   ACT Sign(score + negt8) straight from PSUM -> f16 {+1,-1} mask
         [px, n].  (+1 for top-8, -1 for the rest.)
  maskT  DMA-engine transpose (dma_start_transpose) [px,128n] -> [n,128px]
         per chunk: PE never touches transposes, HAM stays warm.
  matmul outpm[D,px] = sum_n values[n,D]*maskT[n,px]  = 2*S_top8 - S_all
         (fp16 operands, fp32 PSUM accumulation over 16 chunks of n).
  final  ACT: out = outpm*(1/16) + colsum/16  (= mean of top-8) -> DMA out.
"""
import sys

sys.path.insert(0, "/opt/trn_rl_repo")

import numpy as np
import ml_dtypes

B, C, H, W = 32, 3, 64, 64
N, D = 2048, 128
KH = KW = 5
KDIM = C * KH * KW          # 75
KROWS = KDIM + 1            # 76 = patch dims + bias/ones row
NCORES = 8
BLOC = B // NCORES          # 4 images per core
PX = BLOC * H * W           # 16384 pixels per core
GPX = 512                   # pixels per group
NGRP = PX // GPX            # 32 groups per core
NCHUNK = N // 128           # 16 patch chunks
EPS = 1e-5

_CACHE = {}


def _build_program(loop_r=0, ablate=()):
    """loop_r=0: straight-line. loop_r>0: wrap body in a device-side
    For_i loop running it loop_r times (for HW timing via wall deltas).
    ablate: subset of {"dve","act","gpsimd","tp","vmm"} — drop those stages
    (breaks correctness; for timing experiments only)."""
    import concourse.bacc as bacc
    import concourse.tile as tile
    import concourse.mybir as mybir
    from contextlib import ExitStack

    f32 = mybir.dt.float32
    f16 = mybir.dt.float16
    nc = bacc.Bacc("TRN2", target_bir_lowering=False, debug=False)

    xph = nc.dram_tensor("xph", [KROWS, PX], f16, kind="ExternalInput").ap()
    xpl = nc.dram_tensor("xpl", [KROWS, PX], f16, kind="ExternalInput").ap()
    ph = nc.dram_tensor("ph", [KROWS, N], f16, kind="ExternalInput").ap()
    pl = nc.dram_tensor("pl", [KROWS, N], f16, kind="ExternalInput").ap()
    vs16 = nc.dram_tensor("vs16", [128, N], f16, kind="ExternalInput").ap()
    cs16 = nc.dram_tensor("cs16", [128, 1], f32, kind="ExternalInput").ap()
    out = nc.dram_tensor("out", [BLOC, 128, H * W], f32, kind="ExternalOutput").ap()

    SIGN = mybir.ActivationFunctionType.Sign
    IDENT = mybir.ActivationFunctionType.Identity

    with tile.TileContext(nc) as tc, ExitStack() as ctx:
        const = ctx.enter_context(tc.tile_pool(name="const", bufs=1))
        xpp = ctx.enter_context(tc.tile_pool(name="xpp", bufs=4))
        mhp = ctx.enter_context(tc.tile_pool(name="mhp", bufs=4))
        mkp = ctx.enter_context(tc.tile_pool(name="mkp", bufs=6))
        mtp = ctx.enter_context(tc.tile_pool(name="mtp", bufs=2))
        otp = ctx.enter_context(tc.tile_pool(name="otp", bufs=2))
        ps1 = ctx.enter_context(tc.tile_pool(name="ps1", bufs=6, space="PSUM"))
        psB = ctx.enter_context(tc.tile_pool(name="psB", bufs=2, space="PSUM"))

        ph_t = const.tile([KROWS, N], f16)
        pl_t = const.tile([KROWS, N], f16)
        vs_t = const.tile([128, N], f16)
        cs_t = const.tile([128, 1], f32)
        nc.sync.dma_start(ph_t[:], ph[:])
        nc.sync.dma_start(pl_t[:], pl[:])
        nc.sync.dma_start(vs_t[:], vs16[:])
        nc.sync.dma_start(cs_t[:], cs16[:])

        loop_cm = tc.For_i(0, loop_r, 1) if loop_r else None
        if loop_cm is not None:
            loop_cm.__enter__()

        grp_per_img = (H * W) // GPX  # 8

        def pass1_tile(t, xh_t, xl_t, mt):
            lh = xh_t[:, t * 128:(t + 1) * 128]
            ll = xl_t[:, t * 128:(t + 1) * 128]
            mh = mhp.tile([128, 32], f32, tag="mh")
            m8 = mhp.tile([128, 8], f32, tag="m8")
            ng = mhp.tile([128, 1], f32, tag="ng")
            mk = mkp.tile([128, N], f16, tag="mk")
            quads = []
            for q in range(4):  # N=512 per matmul (one PSUM bank)
                rsl = slice(q * 512, (q + 1) * 512)
                p1 = ps1.tile([128, 512], f32, tag="p1")
                nc.tensor.matmul(p1[:], lh, ph_t[:, rsl],
                                 start=True, stop=False)
                nc.tensor.matmul(p1[:], lh, pl_t[:, rsl],
                                 start=False, stop=False)
                nc.tensor.matmul(p1[:], ll, ph_t[:, rsl],
                                 start=False, stop=True)
                if "dve" not in ablate:
                    nc.vector.max(mh[:, q * 8:(q + 1) * 8], p1[:])
                quads.append(p1)
            if "dve" not in ablate:
                nc.vector.max(m8[:], mh[:])
                nc.vector.tensor_scalar(ng[:], m8[:, 7:8], -1.0, EPS,
                                        mybir.AluOpType.mult,
                                        mybir.AluOpType.add)
            if "act" not in ablate:
                for q in range(4):
                    nc.scalar.activation(mk[:, q * 512:(q + 1) * 512],
                                         quads[q][:], SIGN,
                                         bias=ng[:, 0:1], scale=1.0)
            if "tp" not in ablate:
                nc.scalar.dma_start_transpose(
                    mt[:, :, t * 128:(t + 1) * 128], mk[:, :])

        def vchunks(pB, mt, cs):
            if "vmm" in ablate:
                return
            for c in cs:
                rhs = (vs_t[:, 0:GPX] if "tp" in ablate else mt[:, c, :])
                nc.tensor.matmul(
                    pB[:], vs_t[:, c * 128:(c + 1) * 128], rhs,
                    start=(c == 0), stop=(c == NCHUNK - 1))

        def finish(g, pB):
            b, s = divmod(g, grp_per_img)
            ot = otp.tile([128, GPX], f32, tag="ot")
            nc.scalar.activation(ot[:], pB[:], IDENT,
                                 bias=cs_t[:, 0:1], scale=0.0625)
            nc.sync.dma_start(out[b, :, s * GPX:(s + 1) * GPX], ot[:])

        def xload(g):
            xh_t = xpp.tile([KROWS, GPX], f16, tag="xh")
            xl_t = xpp.tile([KROWS, GPX], f16, tag="xl")
            nc.sync.dma_start(xh_t[:], xph[:, g * GPX:(g + 1) * GPX])
            nc.sync.dma_start(xl_t[:], xpl[:, g * GPX:(g + 1) * GPX])
            return xh_t, xl_t

        xts = {0: xload(0)}
        prev = None  # (g-1, pB, mt)
        for g in range(NGRP):
            xh_t, xl_t = xts.pop(g)
            if g + 1 < NGRP:
                xts[g + 1] = xload(g + 1)
            mt = None if "tp" in ablate else mtp.tile([128, NCHUNK, GPX], f16, tag="mt")
            for t in range(4):
                pass1_tile(t, xh_t, xl_t, mt)
            if prev is not None:
                vchunks(prev[1], prev[2], range(NCHUNK))
                finish(prev[0], prev[1])
            pB = psB.tile([128, GPX], f32, tag="pB")
            prev = (g, pB, mt)
        vchunks(prev[1], prev[2], range(NCHUNK))
        finish(prev[0], prev[1])

        if loop_cm is not None:
            loop_cm.__exit__(None, None, None)

    nc.compile()
    return nc


def _get_program():
    if "nc" not in _CACHE:
        _CACHE["nc"] = _build_program()
    return _CACHE["nc"]


def _im2col(x):
    """x: (B,3,64,64) f32 -> cols (B, 75, 4096) f32, k=(c,dy,dx), px=(h,w)."""
    xpad = np.pad(x, ((0, 0), (0, 0), (2, 2), (2, 2)))
    win = np.lib.stride_tricks.sliding_window_view(xpad, (KH, KW), axis=(2, 3))
    cols = np.ascontiguousarray(win.transpose(0, 1, 4, 5, 2, 3))
    return cols.reshape(x.shape[0], KDIM, H * W)


def _host_prep(x, patches, values):
    """Returns per-core in_maps list."""
    pf = patches.reshape(N, KDIM)
    bias = (-0.5 * np.sum(pf.astype(np.float64) ** 2, axis=1)).astype(np.float32)

    pfull = np.zeros((KROWS, N), np.float32)
    pfull[0:KDIM] = pf.T
    pfull[KDIM] = bias
    ph = pfull.astype(np.float16)
    pl = (pfull - ph.astype(np.float32)).astype(np.float16)

    vs16 = np.ascontiguousarray(
        values.reshape(NCHUNK, 128, 128).transpose(1, 0, 2).reshape(128, N)
    ).astype(np.float16)
    # cs16[d] = sum_n values[n, d] / 16 : vs16[r, c*128+d] = values[c*128+r, d]
    cs16 = (vs16.astype(np.float64).reshape(128, NCHUNK, 128).sum(axis=(0, 1))
            / 16.0).astype(np.float32).reshape(128, 1)

    cols = _im2col(x)  # (32, 75, 4096) f32
    in_maps = []
    for i in range(NCORES):
        xfull = np.empty((KROWS, PX), np.float32)
        xfull[0:KDIM] = np.concatenate(
            [cols[i * BLOC + j] for j in range(BLOC)], axis=1)
        xfull[KDIM] = 1.0
        xh = xfull.astype(np.float16)
        xl = (xfull - xh.astype(np.float32)).astype(np.float16)
        in_maps.append({"xph": xh, "xpl": xl, "ph": ph, "pl": pl,
                        "vs16": vs16, "cs16": cs16})
    return in_maps


def kernel(x, patches, values):
    from concourse.bass_utils import run_bass_kernel_spmd

    x = np.asarray(x, dtype=np.float32)
    patches = np.asarray(patches, dtype=np.float32)
    values = np.asarray(values, dtype=np.float32)

    nc = _get_program()
    in_maps = _host_prep(x, patches, values)
    res = run_bass_kernel_spmd(nc, in_maps, list(range(NCORES)))

    out = np.empty((B, D, H, W), np.float32)
    for i in range(NCORES):
        o = res.results[i]["out"]  # (BLOC, 128, 4096)
        out[i * BLOC:(i + 1) * BLOC] = o.reshape(BLOC, D, H, W)
    return out
